# revision 21
# baseline (speedup 1.0000x reference)
"""Trainium2 Bass kernel for nn_Attention (dense_transformer).

Reference computation (H=16 heads, D=1024, DK=64, B=2, S=2048):
    kx = k @ Wk^T + bk ; qx = q @ Wq^T + bq ; vx = k @ Wv^T + bv
    score = einsum('bqhd,bkhd->hbqk', qx, kx) / sqrt(D)
    attn  = softmax(score, -1)                       -> output [H*B, S, S]
    out   = einsum('hbqk,bkhd->bqhd', attn, vx).reshape(B, S, H*DK)
    out   = layernorm(relu(out @ Wd^T + bd)) * g + b -> output [B, S, D]

Sharding: head-parallel across 8 NeuronCores (2 heads/core, both batches).
Launch 1 (per core): transpose q/k on PE, project to qxT/kxT [dk, pos] and
vx [pos, dk] (fp32r matmuls), then per (head, batch) slab:
  path A: scores [q-part, k-free] -> exp(+row-sum accum) -> normalize -> DMA
  path B: scores^T [k-part, q-free] -> exp -> attn@v accumulation in PSUM,
          normalized with path-A sums -> houtT [features, rows]
Launch 2 (row-parallel): dense + bias + relu + layernorm on 512 rows/core.
"""
import os

os.environ.setdefault("JAX_COMPILATION_CACHE_DIR", "/tmp/jax_cache_bass")
os.environ.setdefault("JAX_PERSISTENT_CACHE_MIN_COMPILE_TIME_SECS", "1")

import sys

if "/opt/trn_rl_repo" not in sys.path:
    sys.path.insert(0, "/opt/trn_rl_repo")

import math
from contextlib import ExitStack

import numpy as np

from concourse import bacc, mybir
import concourse.tile as tile
from concourse.bass_utils import run_bass_kernel_spmd

F32 = mybir.dt.float32
F32R = mybir.dt.float32r
AF = mybir.ActivationFunctionType

H, B, S, D, DK = 16, 2, 2048, 1024, 64
NCORES = 8
HPC = H // NCORES            # heads per core = 2
POS = B * S                  # 4096 flattened (b, s) rows
TEMP = math.sqrt(D)          # 32.0
LN_EPS = 1e-5
NSLAB = HPC * B              # 4 (head, batch) slabs per core
QCH = S // 128               # 16 query chunks per slab
KCH = S // 128               # 16 key chunks per slab


def _build_attn_module():
    nc = bacc.Bacc("TRN2", target_bir_lowering=False, debug=False,
                   enable_asserts=True, num_devices=NCORES)

    d_q = nc.dram_tensor("q", (POS, D), F32, kind="ExternalInput").ap()
    d_k = nc.dram_tensor("k", (POS, D), F32, kind="ExternalInput").ap()
    d_wq = nc.dram_tensor("wq_t", (D, HPC * DK), F32, kind="ExternalInput").ap()
    d_wk = nc.dram_tensor("wk_t", (D, HPC * DK), F32, kind="ExternalInput").ap()
    d_wv = nc.dram_tensor("wv_t", (D, HPC * DK), F32, kind="ExternalInput").ap()
    d_bq = nc.dram_tensor("b_q", (HPC * DK, 1), F32, kind="ExternalInput").ap()
    d_bk = nc.dram_tensor("b_k", (HPC * DK, 1), F32, kind="ExternalInput").ap()
    d_bv = nc.dram_tensor("b_v", (HPC * DK, 1), F32, kind="ExternalInput").ap()
    d_id = nc.dram_tensor("ident", (128, 128), F32, kind="ExternalInput").ap()

    d_attn = nc.dram_tensor("attn", (NSLAB, S, S), F32, kind="ExternalOutput").ap()
    d_houtT = nc.dram_tensor("houtT", (HPC * DK, POS), F32,
                             kind="ExternalOutput").ap()

    BF16 = mybir.dt.bfloat16

    with tile.TileContext(nc) as tc, ExitStack() as ctx:
        const = ctx.enter_context(tc.tile_pool(name="const", bufs=1))
        persist = ctx.enter_context(tc.tile_pool(name="persist", bufs=1))
        rows_pool = ctx.enter_context(tc.tile_pool(name="rows", bufs=5))
        qt_pool = ctx.enter_context(tc.tile_pool(name="qtkt", bufs=9))
        vxs_pool = ctx.enter_context(tc.tile_pool(name="vxs", bufs=2))
        e_pool = ctx.enter_context(tc.tile_pool(name="e_sb", bufs=3))
        et_pool = ctx.enter_context(tc.tile_pool(name="et_sb", bufs=3))
        misc = ctx.enter_context(tc.tile_pool(name="misc", bufs=2))
        slabv = ctx.enter_context(tc.tile_pool(name="slabv", bufs=1))
        sc_ps = ctx.enter_context(tc.tile_pool(name="sc_ps", bufs=2, space="PSUM"))
        scb_ps = ctx.enter_context(tc.tile_pool(name="scb_ps", bufs=1, space="PSUM"))
        o_ps = ctx.enter_context(tc.tile_pool(name="o_ps", bufs=1, space="PSUM"))

        t_id = const.tile([128, 128], F32)
        nc.sync.dma_start(t_id[:], d_id)
        # weights: DRAM [1024, 128] -> SBUF [128, 8*128], chunk dc at cols dc*128
        t_wq = const.tile([128, 8 * 128], F32R)
        t_wk = const.tile([128, 8 * 128], F32R)
        t_wv = const.tile([128, 8 * 128], F32R)
        for t_w, d_w in ((t_wq, d_wq), (t_wk, d_wk), (t_wv, d_wv)):
            nc.sync.dma_start(t_w[:].rearrange("p (a n) -> p a n", a=8),
                              d_w.rearrange("(a p) n -> p a n", p=128).bitcast(F32R))
        t_bq = const.tile([128, 1], F32)
        t_bk = const.tile([128, 1], F32)
        nc.sync.dma_start(t_bq[:], d_bq)
        nc.sync.dma_start(t_bk[:], d_bk)
        t_bv = [const.tile([64, 1], F32, tag=f"bv{j}", name=f"t_bv{j}")
                for j in range(HPC)]
        for j in range(HPC):
            nc.sync.dma_start(t_bv[j][:], d_bv[j * 64:(j + 1) * 64, :])
        # K=128 bf16 zero-matmul operand: K=64 matmuls never un-throttle the
        # PE clock gate, so a K=128 burst warms it and sprinkles keep it warm
        t_warm = const.tile([128, 512], BF16)
        nc.gpsimd.memset(t_warm[:], 0.0)

        # persistent activations (partitions 0:64 = head 0, 64:128 = head 1)
        t_qxT = persist.tile([128, POS], F32R)   # [2*dk, pos]
        t_kxT = persist.tile([128, POS], F32R)
        t_vxa = persist.tile([128, POS], F32R)   # slot (j, pb): cols (j*32+pb)*64
        t_houtT = [persist.tile([64, POS], F32, tag=f"houtT{j}",
                                name=f"t_houtT{j}") for j in range(HPC)]
        # zero-masked lhsT staging: score matmuls run K=128 (only rows of the
        # active head are nonzero) because K=64 matmuls leave the PE clock
        # gate throttled at 1.2 GHz
        t_mq = [[persist.tile([128, 128], F32R, tag=f"mq{j}{i}",
                              name=f"t_mq{j}{i}") for i in range(2)]
                for j in range(HPC)]
        t_mk = [[persist.tile([128, 128], F32R, tag=f"mk{j}{i}",
                              name=f"t_mk{j}{i}") for i in range(2)]
                for j in range(HPC)]
        for j in range(HPC):
            for i in range(2):
                nc.gpsimd.memset(t_mq[j][i][:].bitcast(F32), 0.0)
                nc.gpsimd.memset(t_mk[j][i][:].bitcast(F32), 0.0)

        def warm_mm(n=1):
            wp = sc_ps.tile([128, 512], F32, tag="sc", name="wp")
            for _ in range(n):
                nc.tensor.matmul(wp[:], t_warm[:, 0:128], t_warm[:],
                                 start=True, stop=True)

        warm_mm(14)

        def emit_setup_half(b, psl, side):
            """transpose+project one 512-pos slice, one input side (q or k)."""
            p0 = b * S + psl * 512
            if side == "q":
                d_src, t_w, t_bias, dst = d_q, t_wq, t_bq, t_qxT
            else:
                d_src, t_w, t_bias, dst = d_k, t_wk, t_bk, t_kxT
            row_tiles = []
            for i in range(4):
                rt = rows_pool.tile([128, D], F32, tag="rows", name="rt")
                nc.sync.dma_start(rt[:],
                                  d_src[p0 + i * 128: p0 + (i + 1) * 128, :])
                row_tiles.append(rt)
            xts = []
            for dc in range(8):
                tp = sc_ps.tile([128, 512], F32, tag="sc", name="tp")
                for i in range(4):
                    nc.tensor.transpose(
                        tp[:, i * 128:(i + 1) * 128],
                        row_tiles[i][:, dc * 128:(dc + 1) * 128], t_id[:])
                xt = qt_pool.tile([128, 512], F32R, tag="qt", name="xt")
                nc.vector.tensor_copy(xt[:], tp[:])
                xts.append(xt)
            pp = sc_ps.tile([128, 512], F32, tag="sc", name="pp")
            for dc in range(8):
                nc.tensor.matmul(pp[:], t_w[:, dc * 128:(dc + 1) * 128],
                                 xts[dc][:], start=(dc == 0), stop=(dc == 7))
            nc.vector.tensor_scalar(dst[:, p0:p0 + 512], pp[:], t_bias[:],
                                    None, op0=mybir.AluOpType.add)
            if side == "k":
                # v = k: v-projection reuses the k transposes
                pv = sc_ps.tile([128, 512], F32, tag="sc", name="pv")
                for dc in range(8):
                    nc.tensor.matmul(pv[:], t_wv[:, dc * 128:(dc + 1) * 128],
                                     xts[dc][:], start=(dc == 0), stop=(dc == 7))
                vxs = vxs_pool.tile([128, 512], F32, tag="vxs", name="vxs")
                nc.vector.tensor_copy(vxs[:], pv[:])
                # transpose vxT slice -> vx [pos, dk] slots of vxa
                blk0 = p0 // 128
                for j in range(HPC):
                    vp = sc_ps.tile([128, 256], F32, tag="sc", name="vp")
                    for i in range(4):
                        nc.tensor.transpose(
                            vp[:, i * 64:(i + 1) * 64],
                            vxs[j * 64:(j + 1) * 64, i * 128:(i + 1) * 128],
                            t_id[j * 64:(j + 1) * 64, j * 64:(j + 1) * 64])
                    s0 = (j * 32 + blk0) * 64
                    nc.vector.tensor_copy(t_vxa[:, s0:s0 + 256], vp[:])
            warm_mm(1)

        def emit_slab(j, b, weave=None):
            """One (head, batch) slab: 16 merged steps, each = one path-A
            q-chunk (scores->exp->normalize->DMA) + two path-B k-chunk units
            (scores^T->exp->attn@v).  Merging keeps ACT, DMA and PE loaded
            simultaneously; B runs q-half 0 during steps 0-7, half 1 during
            8-15 so each half's accumulator can normalize and free early."""
            weave = weave or {}
            slab = j * B + b
            lo, hi = j * 64, (j + 1) * 64
            c0 = b * S
            sumsA = slabv.tile([128, QCH], F32, tag="sumsA", name="sumsA")
            recA = slabv.tile([128, QCH], F32, tag="recA", name="recA")
            po = [None, None]

            def finish_half(qh):
                # recA cols for this q-half -> [1, 1024] recips -> broadcast
                pt = sc_ps.tile([128, 1024], F32, tag="sc", name="pt")
                nc.tensor.transpose(pt[0:8, 0:128], recA[:, qh * 8:(qh + 1) * 8],
                                    t_id[:])
                rBt = slabv.tile([8, 128], F32, tag="rBt", name="rBt")
                nc.vector.tensor_copy(rBt[:], pt[0:8, 0:128])
                rB = slabv.tile([1, S // 2], F32, tag="rB", name="rB", bufs=2)
                nc.sync.dma_start(
                    rB[0:1, :].rearrange("a (c p) -> a c p", p=128), rBt[:])
                rbB = slabv.tile([64, S // 2], F32, tag="rbB", name="rbB", bufs=2)
                nc.gpsimd.partition_broadcast(rbB[:], rB[0:1, :])
                qb = c0 + qh * 1024
                dst = t_houtT[j][:, qb:qb + S // 2]
                nc.vector.tensor_mul(dst, po[qh][:], rbB[:])
                nc.vector.tensor_scalar(dst, dst, t_bv[j][:], None,
                                        op0=mybir.AluOpType.add)

            for s in range(16):
                qc = s
                qh = s // 8
                if s % 8 == 0:
                    po[qh] = o_ps.tile([64, S // 2], F32, tag="o", name="po")
                # ---- path A chunk ----
                et = e_pool.tile([128, S], F32, tag="E", name="et")
                sh = [misc.tile([128, 1], F32, tag="sh0", name="sh0"),
                      misc.tile([128, 1], F32, tag="sh1", name="sh1")]
                mq = t_mq[j][qc % 2]
                nc.vector.tensor_copy(mq[lo:hi, :],
                                      t_qxT[lo:hi, c0 + qc * 128: c0 + (qc + 1) * 128])
                for h in range(2):
                    ps = sc_ps.tile([128, 1024], F32, tag="sc", name="ps")
                    for ns in range(2):
                        nc.tensor.matmul(
                            ps[:, ns * 512:(ns + 1) * 512],
                            mq[:],
                            t_kxT[:, c0 + h * 1024 + ns * 512:
                                  c0 + h * 1024 + (ns + 1) * 512],
                            start=True, stop=True)
                    nc.scalar.activation(et[:, h * 1024:(h + 1) * 1024], ps[:],
                                         AF.Exp, scale=float(1.0 / TEMP),
                                         accum_out=sh[h][:])
                nc.vector.tensor_add(sumsA[:, qc:qc + 1], sh[0][:], sh[1][:])
                nc.vector.reciprocal(recA[:, qc:qc + 1], sumsA[:, qc:qc + 1])
                nc.vector.tensor_scalar_mul(et[:], et[:], recA[:, qc:qc + 1])
                nc.scalar.dma_start(d_attn[slab, qc * 128:(qc + 1) * 128, :], et[:])

                # ---- two path-B k-chunk units (q-half qh) ----
                qb = c0 + qh * 1024
                for u in range(2):
                    kc = (s % 8) * 2 + u
                    ett = et_pool.tile([128, S // 2], F32R, tag="ET", name="ett")
                    mk = t_mk[j][kc % 2]
                    nc.vector.tensor_copy(
                        mk[lo:hi, :],
                        t_kxT[lo:hi, c0 + kc * 128: c0 + (kc + 1) * 128])
                    ps = scb_ps.tile([128, 1024], F32, tag="scb", name="ps")
                    for ns in range(2):
                        nc.tensor.matmul(
                            ps[:, ns * 512:(ns + 1) * 512],
                            mk[:],
                            t_qxT[:, qb + ns * 512: qb + (ns + 1) * 512],
                            start=True, stop=True)
                    nc.scalar.activation(ett[:], ps[:], AF.Exp,
                                         scale=float(1.0 / TEMP))
                    vslot = (j * 32 + b * 16 + kc) * 64
                    for qs in range(2):
                        nc.tensor.matmul(po[qh][:, qs * 512:(qs + 1) * 512],
                                         t_vxa[:, vslot:vslot + 64],
                                         ett[:, qs * 512:(qs + 1) * 512],
                                         start=(kc == 0), stop=(kc == KCH - 1))

                if s in weave:
                    weave[s]()
                elif s % 2 == 1:
                    warm_mm(1)
                if s % 8 == 7:
                    finish_half(qh)

        for psl in range(4):
            emit_setup_half(0, psl, "k")
            emit_setup_half(0, psl, "q")
        # weave batch-1 setup halves inside batch-0 slab phases so the PE
        # stream keeps dense K=128 work flowing (K=64-only stretches leave
        # the clock gate throttled)
        emit_slab(0, 0,
                  weave={3: lambda: emit_setup_half(1, 0, "k"),
                         7: lambda: emit_setup_half(1, 0, "q"),
                         11: lambda: emit_setup_half(1, 1, "k"),
                         14: lambda: emit_setup_half(1, 1, "q")})
        emit_slab(1, 0,
                  weave={3: lambda: emit_setup_half(1, 2, "k"),
                         7: lambda: emit_setup_half(1, 2, "q"),
                         11: lambda: emit_setup_half(1, 3, "k"),
                         14: lambda: emit_setup_half(1, 3, "q")})
        for j in range(HPC):
            emit_slab(j, 1)

        for j in range(HPC):
            nc.sync.dma_start(d_houtT[j * 64:(j + 1) * 64, :], t_houtT[j][:])

    nc.compile()
    return nc


def _build_dense_module():
    RPC = POS // NCORES      # rows per core = 512
    nc = bacc.Bacc("TRN2", target_bir_lowering=False, debug=False,
                   enable_asserts=True, num_devices=NCORES)

    d_h = nc.dram_tensor("hout_t", (D, RPC), F32, kind="ExternalInput").ap()
    d_w = nc.dram_tensor("dense_wt", (D, D), F32, kind="ExternalInput").ap()
    d_bias = nc.dram_tensor("bias_b", (128, D), F32, kind="ExternalInput").ap()
    d_g = nc.dram_tensor("g_b", (128, D), F32, kind="ExternalInput").ap()
    d_lb = nc.dram_tensor("lb_b", (128, D), F32, kind="ExternalInput").ap()
    d_out = nc.dram_tensor("out2", (RPC, D), F32, kind="ExternalOutput").ap()

    with tile.TileContext(nc) as tc, ExitStack() as ctx:
        const = ctx.enter_context(tc.tile_pool(name="const", bufs=1))
        work = ctx.enter_context(tc.tile_pool(name="work", bufs=2))
        ps_p = ctx.enter_context(tc.tile_pool(name="ps", bufs=2, space="PSUM"))

        t_h = const.tile([128, 8 * RPC], F32R)     # chunk dc at cols dc*512
        nc.sync.dma_start(t_h[:].rearrange("p (a n) -> p a n", a=8),
                          d_h.rearrange("(a p) n -> p a n", p=128).bitcast(F32R))
        t_w = const.tile([128, 8 * D], F32R)       # chunk dc at cols dc*1024
        nc.sync.dma_start(t_w[:].rearrange("p (a n) -> p a n", a=8),
                          d_w.rearrange("(a p) n -> p a n", p=128).bitcast(F32R))
        t_bias = const.tile([128, D], F32)
        t_g = const.tile([128, D], F32)
        t_lb = const.tile([128, D], F32)
        t_eps = const.tile([128, 1], F32)
        nc.gpsimd.memset(t_eps[:], float(LN_EPS))
        nc.sync.dma_start(t_bias[:], d_bias)
        nc.sync.dma_start(t_g[:], d_g)
        nc.sync.dma_start(t_lb[:], d_lb)

        for rc in range(RPC // 128):
            pd = ps_p.tile([128, D], F32, tag="pd")
            for nch in range(2):
                for dc in range(8):
                    nc.tensor.matmul(
                        pd[:, nch * 512:(nch + 1) * 512],
                        t_h[:, dc * RPC + rc * 128: dc * RPC + (rc + 1) * 128],
                        t_w[:, dc * D + nch * 512: dc * D + (nch + 1) * 512],
                        start=(dc == 0), stop=(dc == 7))
            x = work.tile([128, D], F32, tag="x")
            nc.vector.tensor_add(x[:], pd[:], t_bias[:])
            x2 = work.tile([128, D], F32, tag="x2")
            s1 = work.tile([128, 1], F32, tag="s1")
            nc.scalar.activation(x2[:], x[:], AF.Relu, accum_out=s1[:])
            sq = work.tile([128, D], F32, tag="sq")
            s2 = work.tile([128, 1], F32, tag="s2")
            nc.scalar.activation(sq[:], x2[:], AF.Square, accum_out=s2[:])
            mu = work.tile([128, 1], F32, tag="mu")
            nc.vector.tensor_scalar_mul(mu[:], s1[:], float(1.0 / D))
            m2 = work.tile([128, 1], F32, tag="m2")
            nc.vector.tensor_scalar_mul(m2[:], s2[:], float(1.0 / D))
            mu2 = work.tile([128, 1], F32, tag="mu2")
            nc.vector.tensor_mul(mu2[:], mu[:], mu[:])
            var = work.tile([128, 1], F32, tag="var")
            nc.vector.tensor_sub(var[:], m2[:], mu2[:])
            sd = work.tile([128, 1], F32, tag="sd")
            nc.scalar.activation(sd[:], var[:], AF.Sqrt, bias=t_eps[:])
            rstd = work.tile([128, 1], F32, tag="rstd")
            nc.vector.reciprocal(rstd[:], sd[:])
            mb = work.tile([128, 1], F32, tag="mb")
            nc.vector.tensor_mul(mb[:], mu[:], rstd[:])
            xn = work.tile([128, D], F32, tag="xn")
            nc.vector.tensor_scalar(xn[:], x2[:], rstd[:], mb[:],
                                    op0=mybir.AluOpType.mult,
                                    op1=mybir.AluOpType.subtract)
            xg = work.tile([128, D], F32, tag="xg")
            nc.vector.tensor_mul(xg[:], xn[:], t_g[:])
            ot = work.tile([128, D], F32, tag="ot")
            nc.vector.tensor_add(ot[:], xg[:], t_lb[:])
            nc.sync.dma_start(d_out[rc * 128:(rc + 1) * 128, :], ot[:])

    nc.compile()
    return nc


_MODULES = {}
_LAST_IN_MAPS1 = None
_LAST_IN_MAPS2 = None


def _get_modules():
    if "attn" not in _MODULES:
        _MODULES["attn"] = _build_attn_module()
        _MODULES["dense"] = _build_dense_module()
    return _MODULES["attn"], _MODULES["dense"]


def kernel(k, q, w_k_w, w_k_b, w_q_w, w_q_b, w_v_w, w_v_b,
           dense_w, dense_b, ln_g, ln_b):
    k = np.asarray(k, np.float32)
    q = np.asarray(q, np.float32)
    w_k_w = np.asarray(w_k_w, np.float32)
    w_k_b = np.asarray(w_k_b, np.float32)
    w_q_w = np.asarray(w_q_w, np.float32)
    w_q_b = np.asarray(w_q_b, np.float32)
    w_v_w = np.asarray(w_v_w, np.float32)
    w_v_b = np.asarray(w_v_b, np.float32)
    dense_w = np.asarray(dense_w, np.float32)
    dense_b = np.asarray(dense_b, np.float32)
    ln_g = np.asarray(ln_g, np.float32)
    ln_b = np.asarray(ln_b, np.float32)

    nc1, nc2 = _get_modules()

    q2 = np.ascontiguousarray(q.reshape(POS, D))
    k2 = np.ascontiguousarray(k.reshape(POS, D))
    ident = np.eye(128, dtype=np.float32)

    in_maps1 = []
    for c in range(NCORES):
        sl = slice(c * HPC * DK, (c + 1) * HPC * DK)
        in_maps1.append(dict(
            q=q2, k=k2,
            wq_t=np.ascontiguousarray(w_q_w[sl].T),
            wk_t=np.ascontiguousarray(w_k_w[sl].T),
            wv_t=np.ascontiguousarray(w_v_w[sl].T),
            b_q=np.ascontiguousarray(w_q_b[sl].reshape(-1, 1)),
            b_k=np.ascontiguousarray(w_k_b[sl].reshape(-1, 1)),
            b_v=np.ascontiguousarray(w_v_b[sl].reshape(-1, 1)),
            ident=ident,
        ))
    global _LAST_IN_MAPS1
    _LAST_IN_MAPS1 = in_maps1
    res1 = run_bass_kernel_spmd(nc1, in_maps1, core_ids=list(range(NCORES)))
    attn = np.concatenate([r["attn"] for r in res1.results], axis=0)
    houtT = np.concatenate([r["houtT"] for r in res1.results], axis=0)  # [1024, 4096]

    dwt = np.ascontiguousarray(dense_w.T)
    bias_b = np.ascontiguousarray(np.broadcast_to(dense_b, (128, D)))
    g_b = np.ascontiguousarray(np.broadcast_to(ln_g, (128, D)))
    lb_b = np.ascontiguousarray(np.broadcast_to(ln_b, (128, D)))
    RPC = POS // NCORES
    in_maps2 = []
    for c in range(NCORES):
        in_maps2.append(dict(
            hout_t=np.ascontiguousarray(houtT[:, c * RPC:(c + 1) * RPC]),
            dense_wt=dwt, bias_b=bias_b, g_b=g_b, lb_b=lb_b,
        ))
    global _LAST_IN_MAPS2
    _LAST_IN_MAPS2 = in_maps2
    res2 = run_bass_kernel_spmd(nc2, in_maps2, core_ids=list(range(NCORES)))
    out = np.concatenate([r["out2"] for r in res2.results], axis=0).reshape(B, S, D)
    return out, attn


# revision 22
# speedup vs baseline: 1.1282x; 1.1282x over previous
"""Trainium2 Bass kernel for nn_Attention (dense_transformer).

Reference computation (H=16 heads, D=1024, DK=64, B=2, S=2048):
    kx = k @ Wk^T + bk ; qx = q @ Wq^T + bq ; vx = k @ Wv^T + bv
    score = einsum('bqhd,bkhd->hbqk', qx, kx) / sqrt(D)
    attn  = softmax(score, -1)                       -> output [H*B, S, S]
    out   = einsum('hbqk,bkhd->bqhd', attn, vx).reshape(B, S, H*DK)
    out   = layernorm(relu(out @ Wd^T + bd)) * g + b -> output [B, S, D]

Sharding: head-parallel across 8 NeuronCores (2 heads/core, both batches).
Launch 1 (per core): transpose q/k on PE, project to qxT/kxT [dk, pos] and
vx [pos, dk] (fp32r matmuls), then per (head, batch) slab:
  path A: scores [q-part, k-free] -> exp(+row-sum accum) -> normalize -> DMA
  path B: scores^T [k-part, q-free] -> exp -> attn@v accumulation in PSUM,
          normalized with path-A sums -> houtT [features, rows]
Launch 2 (row-parallel): dense + bias + relu + layernorm on 512 rows/core.
"""
import os

os.environ.setdefault("JAX_COMPILATION_CACHE_DIR", "/tmp/jax_cache_bass")
os.environ.setdefault("JAX_PERSISTENT_CACHE_MIN_COMPILE_TIME_SECS", "1")

import sys

if "/opt/trn_rl_repo" not in sys.path:
    sys.path.insert(0, "/opt/trn_rl_repo")

import math
from contextlib import ExitStack

import numpy as np

from concourse import bacc, mybir
import concourse.tile as tile
from concourse.bass_utils import run_bass_kernel_spmd

F32 = mybir.dt.float32
F32R = mybir.dt.float32r
AF = mybir.ActivationFunctionType

H, B, S, D, DK = 16, 2, 2048, 1024, 64
NCORES = 8
HPC = H // NCORES            # heads per core = 2
POS = B * S                  # 4096 flattened (b, s) rows
TEMP = math.sqrt(D)          # 32.0
LN_EPS = 1e-5
NSLAB = HPC * B              # 4 (head, batch) slabs per core
QCH = S // 128               # 16 query chunks per slab
KCH = S // 128               # 16 key chunks per slab


def _build_attn_module():
    nc = bacc.Bacc("TRN2", target_bir_lowering=False, debug=False,
                   enable_asserts=True, num_devices=NCORES)

    d_q = nc.dram_tensor("q", (POS, D), F32, kind="ExternalInput").ap()
    d_k = nc.dram_tensor("k", (POS, D), F32, kind="ExternalInput").ap()
    d_wq = nc.dram_tensor("wq_t", (D, HPC * DK), F32, kind="ExternalInput").ap()
    d_wk = nc.dram_tensor("wk_t", (D, HPC * DK), F32, kind="ExternalInput").ap()
    d_wv = nc.dram_tensor("wv_t", (D, HPC * DK), F32, kind="ExternalInput").ap()
    d_bq = nc.dram_tensor("b_q", (HPC * DK, 1), F32, kind="ExternalInput").ap()
    d_bk = nc.dram_tensor("b_k", (HPC * DK, 1), F32, kind="ExternalInput").ap()
    d_bv = nc.dram_tensor("b_v", (HPC * DK, 1), F32, kind="ExternalInput").ap()
    d_id = nc.dram_tensor("ident", (128, 128), F32, kind="ExternalInput").ap()

    d_attn = nc.dram_tensor("attn", (NSLAB, S, S), F32, kind="ExternalOutput").ap()
    d_houtT = nc.dram_tensor("houtT", (HPC * DK, POS), F32,
                             kind="ExternalOutput").ap()

    BF16 = mybir.dt.bfloat16

    with tile.TileContext(nc) as tc, ExitStack() as ctx:
        const = ctx.enter_context(tc.tile_pool(name="const", bufs=1))
        persist = ctx.enter_context(tc.tile_pool(name="persist", bufs=1))
        rows_pool = ctx.enter_context(tc.tile_pool(name="rows", bufs=5))
        qt_pool = ctx.enter_context(tc.tile_pool(name="qtkt", bufs=9))
        vxs_pool = ctx.enter_context(tc.tile_pool(name="vxs", bufs=2))
        e_pool = ctx.enter_context(tc.tile_pool(name="e_sb", bufs=3))
        et_pool = ctx.enter_context(tc.tile_pool(name="et_sb", bufs=3))
        misc = ctx.enter_context(tc.tile_pool(name="misc", bufs=2))
        slabv = ctx.enter_context(tc.tile_pool(name="slabv", bufs=1))
        sc_ps = ctx.enter_context(tc.tile_pool(name="sc_ps", bufs=2, space="PSUM"))
        scb_ps = ctx.enter_context(tc.tile_pool(name="scb_ps", bufs=1, space="PSUM"))
        o_ps = ctx.enter_context(tc.tile_pool(name="o_ps", bufs=1, space="PSUM"))

        t_id = const.tile([128, 128], F32)
        nc.sync.dma_start(t_id[:], d_id)
        # weights: DRAM [1024, 128] -> SBUF [128, 8*128], chunk dc at cols dc*128
        t_wq = const.tile([128, 8 * 128], F32R)
        t_wk = const.tile([128, 8 * 128], F32R)
        t_wv = const.tile([128, 8 * 128], F32R)
        for t_w, d_w in ((t_wq, d_wq), (t_wk, d_wk), (t_wv, d_wv)):
            nc.sync.dma_start(t_w[:].rearrange("p (a n) -> p a n", a=8),
                              d_w.rearrange("(a p) n -> p a n", p=128).bitcast(F32R))
        t_bq = const.tile([128, 1], F32)
        t_bk = const.tile([128, 1], F32)
        nc.sync.dma_start(t_bq[:], d_bq)
        nc.sync.dma_start(t_bk[:], d_bk)
        t_bv = [const.tile([64, 1], F32, tag=f"bv{j}", name=f"t_bv{j}")
                for j in range(HPC)]
        for j in range(HPC):
            nc.sync.dma_start(t_bv[j][:], d_bv[j * 64:(j + 1) * 64, :])
        # K=128 bf16 zero-matmul operand: K=64 matmuls never un-throttle the
        # PE clock gate, so a K=128 burst warms it and sprinkles keep it warm
        t_warm = const.tile([128, 512], BF16)
        nc.gpsimd.memset(t_warm[:], 0.0)

        # persistent activations (partitions 0:64 = head 0, 64:128 = head 1)
        t_qxT = persist.tile([128, POS], F32R)   # [2*dk, pos]
        t_kxT = persist.tile([128, POS], F32R)
        t_vxa = persist.tile([128, POS], F32R)   # slot (j, pb): cols (j*32+pb)*64
        t_houtT = [persist.tile([64, POS], F32, tag=f"houtT{j}",
                                name=f"t_houtT{j}") for j in range(HPC)]
        # zero-masked lhsT staging: score matmuls run K=128 (only rows of the
        # active head are nonzero) because K=64 matmuls leave the PE clock
        # gate throttled at 1.2 GHz
        t_mq = [[persist.tile([128, 128], F32R, tag=f"mq{j}{i}",
                              name=f"t_mq{j}{i}") for i in range(2)]
                for j in range(HPC)]
        t_mk = [[persist.tile([128, 128], F32R, tag=f"mk{j}{i}",
                              name=f"t_mk{j}{i}") for i in range(2)]
                for j in range(HPC)]
        for j in range(HPC):
            for i in range(2):
                nc.gpsimd.memset(t_mq[j][i][:].bitcast(F32), 0.0)
                nc.gpsimd.memset(t_mk[j][i][:].bitcast(F32), 0.0)

        def warm_mm(n=1):
            wp = sc_ps.tile([128, 512], F32, tag="sc", name="wp")
            for _ in range(n):
                nc.tensor.matmul(wp[:], t_warm[:, 0:128], t_warm[:],
                                 start=True, stop=True)

        warm_mm(14)

        def emit_setup_half(b, psl, side):
            """transpose+project one 512-pos slice, one input side (q or k)."""
            p0 = b * S + psl * 512
            if side == "q":
                d_src, t_w, t_bias, dst = d_q, t_wq, t_bq, t_qxT
            else:
                d_src, t_w, t_bias, dst = d_k, t_wk, t_bk, t_kxT
            row_tiles = []
            for i in range(4):
                rt = rows_pool.tile([128, D], F32, tag="rows", name="rt")
                nc.sync.dma_start(rt[:],
                                  d_src[p0 + i * 128: p0 + (i + 1) * 128, :])
                row_tiles.append(rt)
            xts = []
            for dc in range(8):
                tp = sc_ps.tile([128, 512], F32, tag="sc", name="tp")
                for i in range(4):
                    nc.tensor.transpose(
                        tp[:, i * 128:(i + 1) * 128],
                        row_tiles[i][:, dc * 128:(dc + 1) * 128], t_id[:])
                xt = qt_pool.tile([128, 512], F32R, tag="qt", name="xt")
                nc.vector.tensor_copy(xt[:], tp[:])
                xts.append(xt)
            pp = sc_ps.tile([128, 512], F32, tag="sc", name="pp")
            for dc in range(8):
                nc.tensor.matmul(pp[:], t_w[:, dc * 128:(dc + 1) * 128],
                                 xts[dc][:], start=(dc == 0), stop=(dc == 7))
            nc.vector.tensor_scalar(dst[:, p0:p0 + 512], pp[:], t_bias[:],
                                    None, op0=mybir.AluOpType.add)
            if side == "k":
                # v = k: v-projection reuses the k transposes
                pv = sc_ps.tile([128, 512], F32, tag="sc", name="pv")
                for dc in range(8):
                    nc.tensor.matmul(pv[:], t_wv[:, dc * 128:(dc + 1) * 128],
                                     xts[dc][:], start=(dc == 0), stop=(dc == 7))
                vxs = vxs_pool.tile([128, 512], F32, tag="vxs", name="vxs")
                nc.vector.tensor_copy(vxs[:], pv[:])
                # transpose vxT slice -> vx [pos, dk] slots of vxa
                blk0 = p0 // 128
                for j in range(HPC):
                    vp = sc_ps.tile([128, 256], F32, tag="sc", name="vp")
                    for i in range(4):
                        nc.tensor.transpose(
                            vp[:, i * 64:(i + 1) * 64],
                            vxs[j * 64:(j + 1) * 64, i * 128:(i + 1) * 128],
                            t_id[j * 64:(j + 1) * 64, j * 64:(j + 1) * 64])
                    s0 = (j * 32 + blk0) * 64
                    nc.vector.tensor_copy(t_vxa[:, s0:s0 + 256], vp[:])
            warm_mm(1)

        def emit_slab(j, b, weave=None):
            """One (head, batch) slab: 16 merged steps, each = one path-A
            q-chunk (scores->exp->normalize->DMA) + two path-B k-chunk units
            (scores^T->exp->attn@v).  Merging keeps ACT, DMA and PE loaded
            simultaneously; B runs q-half 0 during steps 0-7, half 1 during
            8-15 so each half's accumulator can normalize and free early."""
            weave = weave or {}
            slab = j * B + b
            lo, hi = j * 64, (j + 1) * 64
            c0 = b * S
            sumsA = slabv.tile([128, QCH], F32, tag="sumsA", name="sumsA")
            recA = slabv.tile([128, QCH], F32, tag="recA", name="recA")
            po = [None, None]

            def finish_half(qh):
                # recA cols for this q-half -> [1, 1024] recips -> broadcast
                pt = sc_ps.tile([128, 1024], F32, tag="sc", name="pt")
                nc.tensor.transpose(pt[0:8, 0:128], recA[:, qh * 8:(qh + 1) * 8],
                                    t_id[:])
                rBt = slabv.tile([8, 128], F32, tag="rBt", name="rBt")
                nc.vector.tensor_copy(rBt[:], pt[0:8, 0:128])
                rB = slabv.tile([1, S // 2], F32, tag="rB", name="rB", bufs=2)
                nc.sync.dma_start(
                    rB[0:1, :].rearrange("a (c p) -> a c p", p=128), rBt[:])
                rbB = slabv.tile([64, S // 2], F32, tag="rbB", name="rbB", bufs=2)
                nc.gpsimd.partition_broadcast(rbB[:], rB[0:1, :])
                qb = c0 + qh * 1024
                dst = t_houtT[j][:, qb:qb + S // 2]
                nc.vector.tensor_mul(dst, po[qh][:], rbB[:])
                nc.vector.tensor_scalar(dst, dst, t_bv[j][:], None,
                                        op0=mybir.AluOpType.add)

            for s in range(16):
                qc = s
                qh = s // 8
                if s % 8 == 0:
                    po[qh] = o_ps.tile([64, S // 2], F32, tag="o", name="po")
                # ---- path A chunk ----
                et = e_pool.tile([128, S], F32, tag="E", name="et")
                sh = [misc.tile([128, 1], F32, tag="sh0", name="sh0"),
                      misc.tile([128, 1], F32, tag="sh1", name="sh1")]
                mq = t_mq[j][qc % 2]
                nc.vector.tensor_copy(mq[lo:hi, :],
                                      t_qxT[lo:hi, c0 + qc * 128: c0 + (qc + 1) * 128])
                for h in range(2):
                    ps = sc_ps.tile([128, 1024], F32, tag="sc", name="ps")
                    for ns in range(2):
                        nc.tensor.matmul(
                            ps[:, ns * 512:(ns + 1) * 512],
                            mq[:],
                            t_kxT[:, c0 + h * 1024 + ns * 512:
                                  c0 + h * 1024 + (ns + 1) * 512],
                            start=True, stop=True)
                    nc.scalar.activation(et[:, h * 1024:(h + 1) * 1024], ps[:],
                                         AF.Exp, scale=float(1.0 / TEMP),
                                         accum_out=sh[h][:])
                nc.vector.tensor_add(sumsA[:, qc:qc + 1], sh[0][:], sh[1][:])
                nc.vector.reciprocal(recA[:, qc:qc + 1], sumsA[:, qc:qc + 1])
                nc.vector.tensor_scalar_mul(et[:], et[:], recA[:, qc:qc + 1])
                nc.sync.dma_start(d_attn[slab, qc * 128:(qc + 1) * 128, :], et[:])

                # ---- two path-B k-chunk units (q-half qh) ----
                qb = c0 + qh * 1024
                for u in range(2):
                    kc = (s % 8) * 2 + u
                    ett = et_pool.tile([128, S // 2], F32R, tag="ET", name="ett")
                    mk = t_mk[j][kc % 2]
                    nc.vector.tensor_copy(
                        mk[lo:hi, :],
                        t_kxT[lo:hi, c0 + kc * 128: c0 + (kc + 1) * 128])
                    ps = scb_ps.tile([128, 1024], F32, tag="scb", name="ps")
                    for ns in range(2):
                        nc.tensor.matmul(
                            ps[:, ns * 512:(ns + 1) * 512],
                            mk[:],
                            t_qxT[:, qb + ns * 512: qb + (ns + 1) * 512],
                            start=True, stop=True)
                    nc.scalar.activation(ett[:], ps[:], AF.Exp,
                                         scale=float(1.0 / TEMP))
                    vslot = (j * 32 + b * 16 + kc) * 64
                    for qs in range(2):
                        nc.tensor.matmul(po[qh][:, qs * 512:(qs + 1) * 512],
                                         t_vxa[:, vslot:vslot + 64],
                                         ett[:, qs * 512:(qs + 1) * 512],
                                         start=(kc == 0), stop=(kc == KCH - 1))

                if s in weave:
                    weave[s]()
                elif s % 2 == 1:
                    warm_mm(1)
                if s % 8 == 7:
                    finish_half(qh)

        for psl in range(4):
            emit_setup_half(0, psl, "k")
            emit_setup_half(0, psl, "q")
        # weave batch-1 setup halves inside batch-0 slab phases so the PE
        # stream keeps dense K=128 work flowing (K=64-only stretches leave
        # the clock gate throttled)
        emit_slab(0, 0,
                  weave={3: lambda: emit_setup_half(1, 0, "k"),
                         7: lambda: emit_setup_half(1, 0, "q"),
                         11: lambda: emit_setup_half(1, 1, "k"),
                         14: lambda: emit_setup_half(1, 1, "q")})
        emit_slab(1, 0,
                  weave={3: lambda: emit_setup_half(1, 2, "k"),
                         7: lambda: emit_setup_half(1, 2, "q"),
                         11: lambda: emit_setup_half(1, 3, "k"),
                         14: lambda: emit_setup_half(1, 3, "q")})
        for j in range(HPC):
            emit_slab(j, 1)

        for j in range(HPC):
            nc.sync.dma_start(d_houtT[j * 64:(j + 1) * 64, :], t_houtT[j][:])

    nc.compile()
    return nc


def _build_dense_module():
    RPC = POS // NCORES      # rows per core = 512
    nc = bacc.Bacc("TRN2", target_bir_lowering=False, debug=False,
                   enable_asserts=True, num_devices=NCORES)

    d_h = nc.dram_tensor("hout_t", (D, RPC), F32, kind="ExternalInput").ap()
    d_w = nc.dram_tensor("dense_wt", (D, D), F32, kind="ExternalInput").ap()
    d_bias = nc.dram_tensor("bias_b", (128, D), F32, kind="ExternalInput").ap()
    d_g = nc.dram_tensor("g_b", (128, D), F32, kind="ExternalInput").ap()
    d_lb = nc.dram_tensor("lb_b", (128, D), F32, kind="ExternalInput").ap()
    d_out = nc.dram_tensor("out2", (RPC, D), F32, kind="ExternalOutput").ap()

    with tile.TileContext(nc) as tc, ExitStack() as ctx:
        const = ctx.enter_context(tc.tile_pool(name="const", bufs=1))
        work = ctx.enter_context(tc.tile_pool(name="work", bufs=2))
        ps_p = ctx.enter_context(tc.tile_pool(name="ps", bufs=2, space="PSUM"))

        t_h = const.tile([128, 8 * RPC], F32R)     # chunk dc at cols dc*512
        nc.sync.dma_start(t_h[:].rearrange("p (a n) -> p a n", a=8),
                          d_h.rearrange("(a p) n -> p a n", p=128).bitcast(F32R))
        t_w = const.tile([128, 8 * D], F32R)       # chunk dc at cols dc*1024
        nc.sync.dma_start(t_w[:].rearrange("p (a n) -> p a n", a=8),
                          d_w.rearrange("(a p) n -> p a n", p=128).bitcast(F32R))
        t_bias = const.tile([128, D], F32)
        t_g = const.tile([128, D], F32)
        t_lb = const.tile([128, D], F32)
        t_eps = const.tile([128, 1], F32)
        nc.gpsimd.memset(t_eps[:], float(LN_EPS))
        nc.sync.dma_start(t_bias[:], d_bias)
        nc.sync.dma_start(t_g[:], d_g)
        nc.sync.dma_start(t_lb[:], d_lb)

        for rc in range(RPC // 128):
            pd = ps_p.tile([128, D], F32, tag="pd")
            for nch in range(2):
                for dc in range(8):
                    nc.tensor.matmul(
                        pd[:, nch * 512:(nch + 1) * 512],
                        t_h[:, dc * RPC + rc * 128: dc * RPC + (rc + 1) * 128],
                        t_w[:, dc * D + nch * 512: dc * D + (nch + 1) * 512],
                        start=(dc == 0), stop=(dc == 7))
            x = work.tile([128, D], F32, tag="x")
            nc.vector.tensor_add(x[:], pd[:], t_bias[:])
            x2 = work.tile([128, D], F32, tag="x2")
            s1 = work.tile([128, 1], F32, tag="s1")
            nc.scalar.activation(x2[:], x[:], AF.Relu, accum_out=s1[:])
            sq = work.tile([128, D], F32, tag="sq")
            s2 = work.tile([128, 1], F32, tag="s2")
            nc.scalar.activation(sq[:], x2[:], AF.Square, accum_out=s2[:])
            mu = work.tile([128, 1], F32, tag="mu")
            nc.vector.tensor_scalar_mul(mu[:], s1[:], float(1.0 / D))
            m2 = work.tile([128, 1], F32, tag="m2")
            nc.vector.tensor_scalar_mul(m2[:], s2[:], float(1.0 / D))
            mu2 = work.tile([128, 1], F32, tag="mu2")
            nc.vector.tensor_mul(mu2[:], mu[:], mu[:])
            var = work.tile([128, 1], F32, tag="var")
            nc.vector.tensor_sub(var[:], m2[:], mu2[:])
            sd = work.tile([128, 1], F32, tag="sd")
            nc.scalar.activation(sd[:], var[:], AF.Sqrt, bias=t_eps[:])
            rstd = work.tile([128, 1], F32, tag="rstd")
            nc.vector.reciprocal(rstd[:], sd[:])
            mb = work.tile([128, 1], F32, tag="mb")
            nc.vector.tensor_mul(mb[:], mu[:], rstd[:])
            xn = work.tile([128, D], F32, tag="xn")
            nc.vector.tensor_scalar(xn[:], x2[:], rstd[:], mb[:],
                                    op0=mybir.AluOpType.mult,
                                    op1=mybir.AluOpType.subtract)
            xg = work.tile([128, D], F32, tag="xg")
            nc.vector.tensor_mul(xg[:], xn[:], t_g[:])
            ot = work.tile([128, D], F32, tag="ot")
            nc.vector.tensor_add(ot[:], xg[:], t_lb[:])
            nc.sync.dma_start(d_out[rc * 128:(rc + 1) * 128, :], ot[:])

    nc.compile()
    return nc


_MODULES = {}
_LAST_IN_MAPS1 = None
_LAST_IN_MAPS2 = None


def _get_modules():
    if "attn" not in _MODULES:
        _MODULES["attn"] = _build_attn_module()
        _MODULES["dense"] = _build_dense_module()
    return _MODULES["attn"], _MODULES["dense"]


def kernel(k, q, w_k_w, w_k_b, w_q_w, w_q_b, w_v_w, w_v_b,
           dense_w, dense_b, ln_g, ln_b):
    k = np.asarray(k, np.float32)
    q = np.asarray(q, np.float32)
    w_k_w = np.asarray(w_k_w, np.float32)
    w_k_b = np.asarray(w_k_b, np.float32)
    w_q_w = np.asarray(w_q_w, np.float32)
    w_q_b = np.asarray(w_q_b, np.float32)
    w_v_w = np.asarray(w_v_w, np.float32)
    w_v_b = np.asarray(w_v_b, np.float32)
    dense_w = np.asarray(dense_w, np.float32)
    dense_b = np.asarray(dense_b, np.float32)
    ln_g = np.asarray(ln_g, np.float32)
    ln_b = np.asarray(ln_b, np.float32)

    nc1, nc2 = _get_modules()

    q2 = np.ascontiguousarray(q.reshape(POS, D))
    k2 = np.ascontiguousarray(k.reshape(POS, D))
    ident = np.eye(128, dtype=np.float32)

    in_maps1 = []
    for c in range(NCORES):
        sl = slice(c * HPC * DK, (c + 1) * HPC * DK)
        in_maps1.append(dict(
            q=q2, k=k2,
            wq_t=np.ascontiguousarray(w_q_w[sl].T),
            wk_t=np.ascontiguousarray(w_k_w[sl].T),
            wv_t=np.ascontiguousarray(w_v_w[sl].T),
            b_q=np.ascontiguousarray(w_q_b[sl].reshape(-1, 1)),
            b_k=np.ascontiguousarray(w_k_b[sl].reshape(-1, 1)),
            b_v=np.ascontiguousarray(w_v_b[sl].reshape(-1, 1)),
            ident=ident,
        ))
    global _LAST_IN_MAPS1
    _LAST_IN_MAPS1 = in_maps1
    res1 = run_bass_kernel_spmd(nc1, in_maps1, core_ids=list(range(NCORES)))
    attn = np.concatenate([r["attn"] for r in res1.results], axis=0)
    houtT = np.concatenate([r["houtT"] for r in res1.results], axis=0)  # [1024, 4096]

    dwt = np.ascontiguousarray(dense_w.T)
    bias_b = np.ascontiguousarray(np.broadcast_to(dense_b, (128, D)))
    g_b = np.ascontiguousarray(np.broadcast_to(ln_g, (128, D)))
    lb_b = np.ascontiguousarray(np.broadcast_to(ln_b, (128, D)))
    RPC = POS // NCORES
    in_maps2 = []
    for c in range(NCORES):
        in_maps2.append(dict(
            hout_t=np.ascontiguousarray(houtT[:, c * RPC:(c + 1) * RPC]),
            dense_wt=dwt, bias_b=bias_b, g_b=g_b, lb_b=lb_b,
        ))
    global _LAST_IN_MAPS2
    _LAST_IN_MAPS2 = in_maps2
    res2 = run_bass_kernel_spmd(nc2, in_maps2, core_ids=list(range(NCORES)))
    out = np.concatenate([r["out2"] for r in res2.results], axis=0).reshape(B, S, D)
    return out, attn


# revision 23
# speedup vs baseline: 1.1466x; 1.0163x over previous
"""Trainium2 Bass kernel for nn_Attention (dense_transformer).

Reference computation (H=16 heads, D=1024, DK=64, B=2, S=2048):
    kx = k @ Wk^T + bk ; qx = q @ Wq^T + bq ; vx = k @ Wv^T + bv
    score = einsum('bqhd,bkhd->hbqk', qx, kx) / sqrt(D)
    attn  = softmax(score, -1)                       -> output [H*B, S, S]
    out   = einsum('hbqk,bkhd->bqhd', attn, vx).reshape(B, S, H*DK)
    out   = layernorm(relu(out @ Wd^T + bd)) * g + b -> output [B, S, D]

Sharding: head-parallel across 8 NeuronCores (2 heads/core, both batches).
Launch 1 (per core): transpose q/k on PE, project to qxT/kxT [dk, pos] and
vx [pos, dk] (fp32r matmuls), then per (head, batch) slab:
  path A: scores [q-part, k-free] -> exp(+row-sum accum) -> normalize -> DMA
  path B: scores^T [k-part, q-free] -> exp -> attn@v accumulation in PSUM,
          normalized with path-A sums -> houtT [features, rows]
Launch 2 (row-parallel): dense + bias + relu + layernorm on 512 rows/core.
"""
import os

os.environ.setdefault("JAX_COMPILATION_CACHE_DIR", "/tmp/jax_cache_bass")
os.environ.setdefault("JAX_PERSISTENT_CACHE_MIN_COMPILE_TIME_SECS", "1")

import sys

if "/opt/trn_rl_repo" not in sys.path:
    sys.path.insert(0, "/opt/trn_rl_repo")

import math
from contextlib import ExitStack

import numpy as np

from concourse import bacc, mybir
import concourse.tile as tile
from concourse.bass_utils import run_bass_kernel_spmd

F32 = mybir.dt.float32
F32R = mybir.dt.float32r
AF = mybir.ActivationFunctionType

H, B, S, D, DK = 16, 2, 2048, 1024, 64
NCORES = 8
HPC = H // NCORES            # heads per core = 2
POS = B * S                  # 4096 flattened (b, s) rows
TEMP = math.sqrt(D)          # 32.0
LN_EPS = 1e-5
NSLAB = HPC * B              # 4 (head, batch) slabs per core
QCH = S // 128               # 16 query chunks per slab
KCH = S // 128               # 16 key chunks per slab


def _build_attn_module():
    nc = bacc.Bacc("TRN2", target_bir_lowering=False, debug=False,
                   enable_asserts=True, num_devices=NCORES)

    d_q = nc.dram_tensor("q", (POS, D), F32, kind="ExternalInput").ap()
    d_k = nc.dram_tensor("k", (POS, D), F32, kind="ExternalInput").ap()
    d_wq = nc.dram_tensor("wq_t", (D, HPC * DK), F32, kind="ExternalInput").ap()
    d_wk = nc.dram_tensor("wk_t", (D, HPC * DK), F32, kind="ExternalInput").ap()
    d_wv = nc.dram_tensor("wv_t", (D, HPC * DK), F32, kind="ExternalInput").ap()
    d_bq = nc.dram_tensor("b_q", (HPC * DK, 1), F32, kind="ExternalInput").ap()
    d_bk = nc.dram_tensor("b_k", (HPC * DK, 1), F32, kind="ExternalInput").ap()
    d_bv = nc.dram_tensor("b_v", (HPC * DK, 1), F32, kind="ExternalInput").ap()
    d_id = nc.dram_tensor("ident", (128, 128), F32, kind="ExternalInput").ap()

    d_attn = nc.dram_tensor("attn", (NSLAB, S, S), F32, kind="ExternalOutput").ap()
    d_houtT = nc.dram_tensor("houtT", (HPC * DK, POS), F32,
                             kind="ExternalOutput").ap()

    BF16 = mybir.dt.bfloat16

    with tile.TileContext(nc) as tc, ExitStack() as ctx:
        const = ctx.enter_context(tc.tile_pool(name="const", bufs=1))
        persist = ctx.enter_context(tc.tile_pool(name="persist", bufs=1))
        rows_pool = ctx.enter_context(tc.tile_pool(name="rows", bufs=5))
        qt_pool = ctx.enter_context(tc.tile_pool(name="qtkt", bufs=9))
        vxs_pool = ctx.enter_context(tc.tile_pool(name="vxs", bufs=2))
        e_pool = ctx.enter_context(tc.tile_pool(name="e_sb", bufs=3))
        et_pool = ctx.enter_context(tc.tile_pool(name="et_sb", bufs=3))
        misc = ctx.enter_context(tc.tile_pool(name="misc", bufs=2))
        slabv = ctx.enter_context(tc.tile_pool(name="slabv", bufs=1))
        sc_ps = ctx.enter_context(tc.tile_pool(name="sc_ps", bufs=2, space="PSUM"))
        scb_ps = ctx.enter_context(tc.tile_pool(name="scb_ps", bufs=1, space="PSUM"))
        o_ps = ctx.enter_context(tc.tile_pool(name="o_ps", bufs=1, space="PSUM"))

        t_id = const.tile([128, 128], F32)
        nc.sync.dma_start(t_id[:], d_id)
        # weights: DRAM [1024, 128] -> SBUF [128, 8*128], chunk dc at cols dc*128
        t_wq = const.tile([128, 8 * 128], F32R)
        t_wk = const.tile([128, 8 * 128], F32R)
        t_wv = const.tile([128, 8 * 128], F32R)
        for t_w, d_w in ((t_wq, d_wq), (t_wk, d_wk), (t_wv, d_wv)):
            nc.sync.dma_start(t_w[:].rearrange("p (a n) -> p a n", a=8),
                              d_w.rearrange("(a p) n -> p a n", p=128).bitcast(F32R))
        t_bq = const.tile([128, 1], F32)
        t_bk = const.tile([128, 1], F32)
        nc.sync.dma_start(t_bq[:], d_bq)
        nc.sync.dma_start(t_bk[:], d_bk)
        t_bv = [const.tile([64, 1], F32, tag=f"bv{j}", name=f"t_bv{j}")
                for j in range(HPC)]
        for j in range(HPC):
            nc.sync.dma_start(t_bv[j][:], d_bv[j * 64:(j + 1) * 64, :])
        # K=128 bf16 zero-matmul operand: K=64 matmuls never un-throttle the
        # PE clock gate, so a K=128 burst warms it and sprinkles keep it warm
        t_warm = const.tile([128, 512], BF16)
        nc.gpsimd.memset(t_warm[:], 0.0)

        # persistent activations (partitions 0:64 = head 0, 64:128 = head 1)
        t_qxT = persist.tile([128, POS], F32R)   # [2*dk, pos]
        t_kxT = persist.tile([128, POS], F32R)
        t_vxa = persist.tile([128, POS], F32R)   # slot (j, pb): cols (j*32+pb)*64
        t_houtT = [persist.tile([64, POS], F32, tag=f"houtT{j}",
                                name=f"t_houtT{j}") for j in range(HPC)]
        # zero-masked lhsT staging: score matmuls run K=128 (only rows of the
        # active head are nonzero) because K=64 matmuls leave the PE clock
        # gate throttled at 1.2 GHz
        t_mq = [[persist.tile([128, 128], F32R, tag=f"mq{j}{i}",
                              name=f"t_mq{j}{i}") for i in range(2)]
                for j in range(HPC)]
        t_mk = [[persist.tile([128, 128], F32R, tag=f"mk{j}{i}",
                              name=f"t_mk{j}{i}") for i in range(2)]
                for j in range(HPC)]
        for j in range(HPC):
            for i in range(2):
                nc.gpsimd.memset(t_mq[j][i][:].bitcast(F32), 0.0)
                nc.gpsimd.memset(t_mk[j][i][:].bitcast(F32), 0.0)

        def warm_mm(n=1):
            wp = sc_ps.tile([128, 512], F32, tag="sc", name="wp")
            for _ in range(n):
                nc.tensor.matmul(wp[:], t_warm[:, 0:128], t_warm[:],
                                 start=True, stop=True)

        warm_mm(14)

        def emit_setup_half(b, psl, side):
            """transpose+project one 512-pos slice, one input side (q or k)."""
            p0 = b * S + psl * 512
            if side == "q":
                d_src, t_w, t_bias, dst = d_q, t_wq, t_bq, t_qxT
            else:
                d_src, t_w, t_bias, dst = d_k, t_wk, t_bk, t_kxT
            row_tiles = []
            for i in range(4):
                rt = rows_pool.tile([128, D], F32, tag="rows", name="rt")
                nc.sync.dma_start(rt[:],
                                  d_src[p0 + i * 128: p0 + (i + 1) * 128, :])
                row_tiles.append(rt)
            xts = []
            for dc in range(8):
                tp = sc_ps.tile([128, 512], F32, tag="sc", name="tp")
                for i in range(4):
                    nc.tensor.transpose(
                        tp[:, i * 128:(i + 1) * 128],
                        row_tiles[i][:, dc * 128:(dc + 1) * 128], t_id[:])
                xt = qt_pool.tile([128, 512], F32R, tag="qt", name="xt")
                nc.vector.tensor_copy(xt[:], tp[:])
                xts.append(xt)
            pp = sc_ps.tile([128, 512], F32, tag="sc", name="pp")
            for dc in range(8):
                nc.tensor.matmul(pp[:], t_w[:, dc * 128:(dc + 1) * 128],
                                 xts[dc][:], start=(dc == 0), stop=(dc == 7))
            nc.vector.tensor_scalar(dst[:, p0:p0 + 512], pp[:], t_bias[:],
                                    None, op0=mybir.AluOpType.add)
            if side == "k":
                # v = k: v-projection reuses the k transposes
                pv = sc_ps.tile([128, 512], F32, tag="sc", name="pv")
                for dc in range(8):
                    nc.tensor.matmul(pv[:], t_wv[:, dc * 128:(dc + 1) * 128],
                                     xts[dc][:], start=(dc == 0), stop=(dc == 7))
                vxs = vxs_pool.tile([128, 512], F32, tag="vxs", name="vxs")
                nc.vector.tensor_copy(vxs[:], pv[:])
                # transpose vxT slice -> vx [pos, dk] slots of vxa
                blk0 = p0 // 128
                for j in range(HPC):
                    vp = sc_ps.tile([128, 256], F32, tag="sc", name="vp")
                    for i in range(4):
                        nc.tensor.transpose(
                            vp[:, i * 64:(i + 1) * 64],
                            vxs[j * 64:(j + 1) * 64, i * 128:(i + 1) * 128],
                            t_id[j * 64:(j + 1) * 64, j * 64:(j + 1) * 64])
                    s0 = (j * 32 + blk0) * 64
                    nc.vector.tensor_copy(t_vxa[:, s0:s0 + 256], vp[:])
            warm_mm(1)

        def emit_slab(j, b, weave=None):
            """One (head, batch) slab: 16 merged steps, each = one path-A
            q-chunk (scores->exp->normalize->DMA) + two path-B k-chunk units
            (scores^T->exp->attn@v).  Merging keeps ACT, DMA and PE loaded
            simultaneously; B runs q-half 0 during steps 0-7, half 1 during
            8-15 so each half's accumulator can normalize and free early."""
            weave = weave or {}
            slab = j * B + b
            lo, hi = j * 64, (j + 1) * 64
            c0 = b * S
            sumsA = slabv.tile([128, QCH], F32, tag="sumsA", name="sumsA")
            recA = slabv.tile([128, QCH], F32, tag="recA", name="recA")
            po = [None, None]

            def finish_half(qh):
                # recA cols for this q-half -> [1, 1024] recips -> broadcast
                pt = sc_ps.tile([128, 1024], F32, tag="sc", name="pt")
                nc.tensor.transpose(pt[0:8, 0:128], recA[:, qh * 8:(qh + 1) * 8],
                                    t_id[:])
                rBt = slabv.tile([8, 128], F32, tag="rBt", name="rBt")
                nc.vector.tensor_copy(rBt[:], pt[0:8, 0:128])
                rB = slabv.tile([1, S // 2], F32, tag="rB", name="rB", bufs=2)
                nc.sync.dma_start(
                    rB[0:1, :].rearrange("a (c p) -> a c p", p=128), rBt[:])
                rbB = slabv.tile([64, S // 2], F32, tag="rbB", name="rbB", bufs=2)
                nc.gpsimd.partition_broadcast(rbB[:], rB[0:1, :])
                qb = c0 + qh * 1024
                dst = t_houtT[j][:, qb:qb + S // 2]
                nc.vector.tensor_mul(dst, po[qh][:], rbB[:])
                nc.vector.tensor_scalar(dst, dst, t_bv[j][:], None,
                                        op0=mybir.AluOpType.add)

            for s in range(16):
                qc = s
                qh = s // 8
                if s % 8 == 0:
                    po[qh] = o_ps.tile([64, S // 2], F32, tag="o", name="po")
                # ---- path A chunk ----
                et = e_pool.tile([128, S], F32, tag="E", name="et")
                sh = [misc.tile([128, 1], F32, tag="sh0", name="sh0"),
                      misc.tile([128, 1], F32, tag="sh1", name="sh1")]
                mq = t_mq[j][qc % 2]
                nc.vector.tensor_copy(mq[lo:hi, :],
                                      t_qxT[lo:hi, c0 + qc * 128: c0 + (qc + 1) * 128])
                for h in range(2):
                    ps = sc_ps.tile([128, 1024], F32, tag="sc", name="ps")
                    for ns in range(2):
                        nc.tensor.matmul(
                            ps[:, ns * 512:(ns + 1) * 512],
                            mq[:],
                            t_kxT[:, c0 + h * 1024 + ns * 512:
                                  c0 + h * 1024 + (ns + 1) * 512],
                            start=True, stop=True)
                    nc.scalar.activation(et[:, h * 1024:(h + 1) * 1024], ps[:],
                                         AF.Exp, scale=float(1.0 / TEMP),
                                         accum_out=sh[h][:])
                nc.vector.tensor_add(sumsA[:, qc:qc + 1], sh[0][:], sh[1][:])
                nc.vector.reciprocal(recA[:, qc:qc + 1], sumsA[:, qc:qc + 1])
                nc.vector.tensor_scalar_mul(et[:], et[:], recA[:, qc:qc + 1])
                nc.sync.dma_start(d_attn[slab, qc * 128:(qc + 1) * 128, :], et[:])

                # ---- two path-B k-chunk units (q-half qh) ----
                qb = c0 + qh * 1024
                for u in range(2):
                    kc = (s % 8) * 2 + u
                    ett = et_pool.tile([128, S // 2], F32R, tag="ET", name="ett")
                    mk = t_mk[j][kc % 2]
                    nc.vector.tensor_copy(
                        mk[lo:hi, :],
                        t_kxT[lo:hi, c0 + kc * 128: c0 + (kc + 1) * 128])
                    ps = scb_ps.tile([128, 1024], F32, tag="scb", name="ps")
                    for ns in range(2):
                        nc.tensor.matmul(
                            ps[:, ns * 512:(ns + 1) * 512],
                            mk[:],
                            t_qxT[:, qb + ns * 512: qb + (ns + 1) * 512],
                            start=True, stop=True)
                    nc.scalar.activation(ett[:], ps[:], AF.Exp,
                                         scale=float(1.0 / TEMP))
                    vslot = (j * 32 + b * 16 + kc) * 64
                    for qs in range(2):
                        nc.tensor.matmul(po[qh][:, qs * 512:(qs + 1) * 512],
                                         t_vxa[:, vslot:vslot + 64],
                                         ett[:, qs * 512:(qs + 1) * 512],
                                         start=(kc == 0), stop=(kc == KCH - 1))

                if s in weave:
                    weave[s]()
                elif s % 2 == 1:
                    warm_mm(1)
                if s % 8 == 7:
                    finish_half(qh)

        # minimal prefix: slab(0,0) needs all batch-0 k-slices, q-slices 0-1
        # (for A-chunks 0-7 and the B q-half 0); the rest weaves into slabs
        # so ACT starts exp work as early as possible
        for psl in range(4):
            emit_setup_half(0, psl, "k")
        emit_setup_half(0, 0, "q")
        emit_setup_half(0, 1, "q")
        emit_slab(0, 0,
                  weave={1: lambda: emit_setup_half(0, 2, "q"),
                         4: lambda: emit_setup_half(0, 3, "q"),
                         7: lambda: emit_setup_half(1, 0, "k"),
                         10: lambda: emit_setup_half(1, 0, "q"),
                         12: lambda: emit_setup_half(1, 1, "k"),
                         14: lambda: emit_setup_half(1, 1, "q")})
        emit_slab(1, 0,
                  weave={3: lambda: emit_setup_half(1, 2, "k"),
                         7: lambda: emit_setup_half(1, 2, "q"),
                         11: lambda: emit_setup_half(1, 3, "k"),
                         14: lambda: emit_setup_half(1, 3, "q")})
        for j in range(HPC):
            emit_slab(j, 1)

        for j in range(HPC):
            nc.sync.dma_start(d_houtT[j * 64:(j + 1) * 64, :], t_houtT[j][:])

    nc.compile()
    return nc


def _build_dense_module():
    RPC = POS // NCORES      # rows per core = 512
    nc = bacc.Bacc("TRN2", target_bir_lowering=False, debug=False,
                   enable_asserts=True, num_devices=NCORES)

    d_h = nc.dram_tensor("hout_t", (D, RPC), F32, kind="ExternalInput").ap()
    d_w = nc.dram_tensor("dense_wt", (D, D), F32, kind="ExternalInput").ap()
    d_bias = nc.dram_tensor("bias_b", (128, D), F32, kind="ExternalInput").ap()
    d_g = nc.dram_tensor("g_b", (128, D), F32, kind="ExternalInput").ap()
    d_lb = nc.dram_tensor("lb_b", (128, D), F32, kind="ExternalInput").ap()
    d_out = nc.dram_tensor("out2", (RPC, D), F32, kind="ExternalOutput").ap()

    with tile.TileContext(nc) as tc, ExitStack() as ctx:
        const = ctx.enter_context(tc.tile_pool(name="const", bufs=1))
        work = ctx.enter_context(tc.tile_pool(name="work", bufs=2))
        ps_p = ctx.enter_context(tc.tile_pool(name="ps", bufs=2, space="PSUM"))

        t_h = const.tile([128, 8 * RPC], F32R)     # chunk dc at cols dc*512
        nc.sync.dma_start(t_h[:].rearrange("p (a n) -> p a n", a=8),
                          d_h.rearrange("(a p) n -> p a n", p=128).bitcast(F32R))
        t_w = const.tile([128, 8 * D], F32R)       # chunk dc at cols dc*1024
        nc.sync.dma_start(t_w[:].rearrange("p (a n) -> p a n", a=8),
                          d_w.rearrange("(a p) n -> p a n", p=128).bitcast(F32R))
        t_bias = const.tile([128, D], F32)
        t_g = const.tile([128, D], F32)
        t_lb = const.tile([128, D], F32)
        t_eps = const.tile([128, 1], F32)
        nc.gpsimd.memset(t_eps[:], float(LN_EPS))
        nc.sync.dma_start(t_bias[:], d_bias)
        nc.sync.dma_start(t_g[:], d_g)
        nc.sync.dma_start(t_lb[:], d_lb)

        for rc in range(RPC // 128):
            pd = ps_p.tile([128, D], F32, tag="pd")
            for nch in range(2):
                for dc in range(8):
                    nc.tensor.matmul(
                        pd[:, nch * 512:(nch + 1) * 512],
                        t_h[:, dc * RPC + rc * 128: dc * RPC + (rc + 1) * 128],
                        t_w[:, dc * D + nch * 512: dc * D + (nch + 1) * 512],
                        start=(dc == 0), stop=(dc == 7))
            x = work.tile([128, D], F32, tag="x")
            nc.vector.tensor_add(x[:], pd[:], t_bias[:])
            x2 = work.tile([128, D], F32, tag="x2")
            s1 = work.tile([128, 1], F32, tag="s1")
            nc.scalar.activation(x2[:], x[:], AF.Relu, accum_out=s1[:])
            sq = work.tile([128, D], F32, tag="sq")
            s2 = work.tile([128, 1], F32, tag="s2")
            nc.scalar.activation(sq[:], x2[:], AF.Square, accum_out=s2[:])
            mu = work.tile([128, 1], F32, tag="mu")
            nc.vector.tensor_scalar_mul(mu[:], s1[:], float(1.0 / D))
            m2 = work.tile([128, 1], F32, tag="m2")
            nc.vector.tensor_scalar_mul(m2[:], s2[:], float(1.0 / D))
            mu2 = work.tile([128, 1], F32, tag="mu2")
            nc.vector.tensor_mul(mu2[:], mu[:], mu[:])
            var = work.tile([128, 1], F32, tag="var")
            nc.vector.tensor_sub(var[:], m2[:], mu2[:])
            sd = work.tile([128, 1], F32, tag="sd")
            nc.scalar.activation(sd[:], var[:], AF.Sqrt, bias=t_eps[:])
            rstd = work.tile([128, 1], F32, tag="rstd")
            nc.vector.reciprocal(rstd[:], sd[:])
            mb = work.tile([128, 1], F32, tag="mb")
            nc.vector.tensor_mul(mb[:], mu[:], rstd[:])
            xn = work.tile([128, D], F32, tag="xn")
            nc.vector.tensor_scalar(xn[:], x2[:], rstd[:], mb[:],
                                    op0=mybir.AluOpType.mult,
                                    op1=mybir.AluOpType.subtract)
            xg = work.tile([128, D], F32, tag="xg")
            nc.vector.tensor_mul(xg[:], xn[:], t_g[:])
            ot = work.tile([128, D], F32, tag="ot")
            nc.vector.tensor_add(ot[:], xg[:], t_lb[:])
            nc.sync.dma_start(d_out[rc * 128:(rc + 1) * 128, :], ot[:])

    nc.compile()
    return nc


_MODULES = {}
_LAST_IN_MAPS1 = None
_LAST_IN_MAPS2 = None


def _get_modules():
    if "attn" not in _MODULES:
        _MODULES["attn"] = _build_attn_module()
        _MODULES["dense"] = _build_dense_module()
    return _MODULES["attn"], _MODULES["dense"]


def kernel(k, q, w_k_w, w_k_b, w_q_w, w_q_b, w_v_w, w_v_b,
           dense_w, dense_b, ln_g, ln_b):
    k = np.asarray(k, np.float32)
    q = np.asarray(q, np.float32)
    w_k_w = np.asarray(w_k_w, np.float32)
    w_k_b = np.asarray(w_k_b, np.float32)
    w_q_w = np.asarray(w_q_w, np.float32)
    w_q_b = np.asarray(w_q_b, np.float32)
    w_v_w = np.asarray(w_v_w, np.float32)
    w_v_b = np.asarray(w_v_b, np.float32)
    dense_w = np.asarray(dense_w, np.float32)
    dense_b = np.asarray(dense_b, np.float32)
    ln_g = np.asarray(ln_g, np.float32)
    ln_b = np.asarray(ln_b, np.float32)

    nc1, nc2 = _get_modules()

    q2 = np.ascontiguousarray(q.reshape(POS, D))
    k2 = np.ascontiguousarray(k.reshape(POS, D))
    ident = np.eye(128, dtype=np.float32)

    in_maps1 = []
    for c in range(NCORES):
        sl = slice(c * HPC * DK, (c + 1) * HPC * DK)
        in_maps1.append(dict(
            q=q2, k=k2,
            wq_t=np.ascontiguousarray(w_q_w[sl].T),
            wk_t=np.ascontiguousarray(w_k_w[sl].T),
            wv_t=np.ascontiguousarray(w_v_w[sl].T),
            b_q=np.ascontiguousarray(w_q_b[sl].reshape(-1, 1)),
            b_k=np.ascontiguousarray(w_k_b[sl].reshape(-1, 1)),
            b_v=np.ascontiguousarray(w_v_b[sl].reshape(-1, 1)),
            ident=ident,
        ))
    global _LAST_IN_MAPS1
    _LAST_IN_MAPS1 = in_maps1
    res1 = run_bass_kernel_spmd(nc1, in_maps1, core_ids=list(range(NCORES)))
    attn = np.concatenate([r["attn"] for r in res1.results], axis=0)
    houtT = np.concatenate([r["houtT"] for r in res1.results], axis=0)  # [1024, 4096]

    dwt = np.ascontiguousarray(dense_w.T)
    bias_b = np.ascontiguousarray(np.broadcast_to(dense_b, (128, D)))
    g_b = np.ascontiguousarray(np.broadcast_to(ln_g, (128, D)))
    lb_b = np.ascontiguousarray(np.broadcast_to(ln_b, (128, D)))
    RPC = POS // NCORES
    in_maps2 = []
    for c in range(NCORES):
        in_maps2.append(dict(
            hout_t=np.ascontiguousarray(houtT[:, c * RPC:(c + 1) * RPC]),
            dense_wt=dwt, bias_b=bias_b, g_b=g_b, lb_b=lb_b,
        ))
    global _LAST_IN_MAPS2
    _LAST_IN_MAPS2 = in_maps2
    res2 = run_bass_kernel_spmd(nc2, in_maps2, core_ids=list(range(NCORES)))
    out = np.concatenate([r["out2"] for r in res2.results], axis=0).reshape(B, S, D)
    return out, attn


# revision 24
# speedup vs baseline: 1.1984x; 1.0452x over previous
"""Trainium2 Bass kernel for nn_Attention (dense_transformer).

Reference computation (H=16 heads, D=1024, DK=64, B=2, S=2048):
    kx = k @ Wk^T + bk ; qx = q @ Wq^T + bq ; vx = k @ Wv^T + bv
    score = einsum('bqhd,bkhd->hbqk', qx, kx) / sqrt(D)
    attn  = softmax(score, -1)                       -> output [H*B, S, S]
    out   = einsum('hbqk,bkhd->bqhd', attn, vx).reshape(B, S, H*DK)
    out   = layernorm(relu(out @ Wd^T + bd)) * g + b -> output [B, S, D]

Sharding: head-parallel across 8 NeuronCores (2 heads/core, both batches).
Launch 1 (per core): transpose q/k on PE, project to qxT/kxT [dk, pos] and
vx [pos, dk] (fp32r matmuls), then per (head, batch) slab:
  path A: scores [q-part, k-free] -> exp(+row-sum accum) -> normalize -> DMA
  path B: scores^T [k-part, q-free] -> exp -> attn@v accumulation in PSUM,
          normalized with path-A sums -> houtT [features, rows]
Launch 2 (row-parallel): dense + bias + relu + layernorm on 512 rows/core.
"""
import os

os.environ.setdefault("JAX_COMPILATION_CACHE_DIR", "/tmp/jax_cache_bass")
os.environ.setdefault("JAX_PERSISTENT_CACHE_MIN_COMPILE_TIME_SECS", "1")

import sys

if "/opt/trn_rl_repo" not in sys.path:
    sys.path.insert(0, "/opt/trn_rl_repo")

import math
from contextlib import ExitStack

import numpy as np

from concourse import bacc, mybir
import concourse.tile as tile
from concourse.bass_utils import run_bass_kernel_spmd

F32 = mybir.dt.float32
F32R = mybir.dt.float32r
AF = mybir.ActivationFunctionType

H, B, S, D, DK = 16, 2, 2048, 1024, 64
NCORES = 8
HPC = H // NCORES            # heads per core = 2
POS = B * S                  # 4096 flattened (b, s) rows
TEMP = math.sqrt(D)          # 32.0
LN_EPS = 1e-5
NSLAB = HPC * B              # 4 (head, batch) slabs per core
QCH = S // 128               # 16 query chunks per slab
KCH = S // 128               # 16 key chunks per slab


def _build_attn_module():
    nc = bacc.Bacc("TRN2", target_bir_lowering=False, debug=False,
                   enable_asserts=True, num_devices=NCORES)

    d_q = nc.dram_tensor("q", (POS, D), F32, kind="ExternalInput").ap()
    d_k = nc.dram_tensor("k", (POS, D), F32, kind="ExternalInput").ap()
    d_wq = nc.dram_tensor("wq_t", (D, HPC * DK), F32, kind="ExternalInput").ap()
    d_wk = nc.dram_tensor("wk_t", (D, HPC * DK), F32, kind="ExternalInput").ap()
    d_wv = nc.dram_tensor("wv_t", (D, HPC * DK), F32, kind="ExternalInput").ap()
    d_bq = nc.dram_tensor("b_q", (HPC * DK, 1), F32, kind="ExternalInput").ap()
    d_bk = nc.dram_tensor("b_k", (HPC * DK, 1), F32, kind="ExternalInput").ap()
    d_bv = nc.dram_tensor("b_v", (HPC * DK, 1), F32, kind="ExternalInput").ap()
    d_id = nc.dram_tensor("ident", (128, 128), F32, kind="ExternalInput").ap()

    d_attn = nc.dram_tensor("attn", (NSLAB, S, S), F32, kind="ExternalOutput").ap()
    d_houtT = nc.dram_tensor("houtT", (HPC * DK, POS), F32,
                             kind="ExternalOutput").ap()

    BF16 = mybir.dt.bfloat16

    with tile.TileContext(nc) as tc, ExitStack() as ctx:
        const = ctx.enter_context(tc.tile_pool(name="const", bufs=1))
        persist = ctx.enter_context(tc.tile_pool(name="persist", bufs=1))
        rows_pool = ctx.enter_context(tc.tile_pool(name="rows", bufs=5))
        qt_pool = ctx.enter_context(tc.tile_pool(name="qtkt", bufs=5))
        vxs_pool = ctx.enter_context(tc.tile_pool(name="vxs", bufs=2))
        e_pool = ctx.enter_context(tc.tile_pool(name="e_sb", bufs=3))
        et_pool = ctx.enter_context(tc.tile_pool(name="et_sb", bufs=3))
        misc = ctx.enter_context(tc.tile_pool(name="misc", bufs=2))
        slabv = ctx.enter_context(tc.tile_pool(name="slabv", bufs=1))
        sc_ps = ctx.enter_context(tc.tile_pool(name="sc_ps", bufs=2, space="PSUM"))
        scb_ps = ctx.enter_context(tc.tile_pool(name="scb_ps", bufs=1, space="PSUM"))
        o_ps = ctx.enter_context(tc.tile_pool(name="o_ps", bufs=1, space="PSUM"))

        t_id = const.tile([128, 128], F32)
        nc.sync.dma_start(t_id[:], d_id)
        # weights: DRAM [1024, 128] -> SBUF [128, 8*128], chunk dc at cols dc*128
        t_wq = const.tile([128, 8 * 128], F32R)
        t_wk = const.tile([128, 8 * 128], F32R)
        t_wv = const.tile([128, 8 * 128], F32R)
        for t_w, d_w in ((t_wq, d_wq), (t_wk, d_wk), (t_wv, d_wv)):
            nc.sync.dma_start(t_w[:].rearrange("p (a n) -> p a n", a=8),
                              d_w.rearrange("(a p) n -> p a n", p=128).bitcast(F32R))
        t_bq = const.tile([128, 1], F32)
        t_bk = const.tile([128, 1], F32)
        nc.sync.dma_start(t_bq[:], d_bq)
        nc.sync.dma_start(t_bk[:], d_bk)
        t_bv = [const.tile([64, 1], F32, tag=f"bv{j}", name=f"t_bv{j}")
                for j in range(HPC)]
        for j in range(HPC):
            nc.sync.dma_start(t_bv[j][:], d_bv[j * 64:(j + 1) * 64, :])
        # K=128 bf16 zero-matmul operand: K=64 matmuls never un-throttle the
        # PE clock gate, so a K=128 burst warms it and sprinkles keep it warm
        t_warm = const.tile([128, 512], BF16)
        nc.gpsimd.memset(t_warm[:], 0.0)

        # persistent activations (partitions 0:64 = head 0, 64:128 = head 1)
        t_qxT = persist.tile([128, POS], F32R)   # [2*dk, pos]
        t_kxT = persist.tile([128, POS], F32R)
        t_vxa = persist.tile([128, POS], F32R)   # slot (j, pb): cols (j*32+pb)*64
        t_houtT = [persist.tile([64, POS], F32, tag=f"houtT{j}",
                                name=f"t_houtT{j}") for j in range(HPC)]
        # zero-masked lhsT staging: score matmuls run K=128 (only rows of the
        # active head are nonzero) because K=64 matmuls leave the PE clock
        # gate throttled at 1.2 GHz
        t_mq = [[persist.tile([128, 128], F32R, tag=f"mq{j}{i}",
                              name=f"t_mq{j}{i}") for i in range(2)]
                for j in range(HPC)]
        t_mk = [[persist.tile([128, 128], F32R, tag=f"mk{j}{i}",
                              name=f"t_mk{j}{i}") for i in range(2)]
                for j in range(HPC)]
        for j in range(HPC):
            for i in range(2):
                nc.gpsimd.memset(t_mq[j][i][:].bitcast(F32), 0.0)
                nc.gpsimd.memset(t_mk[j][i][:].bitcast(F32), 0.0)

        def warm_mm(n=1):
            wp = sc_ps.tile([128, 512], F32, tag="sc", name="wp")
            for _ in range(n):
                nc.tensor.matmul(wp[:], t_warm[:, 0:128], t_warm[:],
                                 start=True, stop=True)

        warm_mm(14)

        def emit_setup_half(b, psl, side):
            """transpose+project one 512-pos slice, one input side (q or k)."""
            p0 = b * S + psl * 512
            if side == "q":
                d_src, t_w, t_bias, dst = d_q, t_wq, t_bq, t_qxT
            else:
                d_src, t_w, t_bias, dst = d_k, t_wk, t_bk, t_kxT
            row_tiles = []
            for i in range(4):
                rt = rows_pool.tile([128, D], F32, tag="rows", name="rt")
                nc.sync.dma_start(rt[:],
                                  d_src[p0 + i * 128: p0 + (i + 1) * 128, :])
                row_tiles.append(rt)
            xt2 = []
            for dc2 in range(4):
                tp = sc_ps.tile([128, 1024], F32, tag="sc", name="tp")
                for sub in range(2):
                    dc = dc2 * 2 + sub
                    for i in range(4):
                        nc.tensor.transpose(
                            tp[:, sub * 512 + i * 128: sub * 512 + (i + 1) * 128],
                            row_tiles[i][:, dc * 128:(dc + 1) * 128], t_id[:])
                xt = qt_pool.tile([128, 1024], F32R, tag="qt", name="xt")
                nc.vector.tensor_copy(xt[:], tp[:])
                xt2.append(xt)
            xts = [xt2[dc // 2][:, (dc % 2) * 512:((dc % 2) + 1) * 512]
                   for dc in range(8)]
            pp = sc_ps.tile([128, 512], F32, tag="sc", name="pp")
            for dc in range(8):
                nc.tensor.matmul(pp[:], t_w[:, dc * 128:(dc + 1) * 128],
                                 xts[dc], start=(dc == 0), stop=(dc == 7))
            nc.vector.tensor_scalar(dst[:, p0:p0 + 512], pp[:], t_bias[:],
                                    None, op0=mybir.AluOpType.add)
            if side == "k":
                # v = k: v-projection reuses the k transposes
                pv = sc_ps.tile([128, 512], F32, tag="sc", name="pv")
                for dc in range(8):
                    nc.tensor.matmul(pv[:], t_wv[:, dc * 128:(dc + 1) * 128],
                                     xts[dc], start=(dc == 0), stop=(dc == 7))
                vxs = vxs_pool.tile([128, 512], F32, tag="vxs", name="vxs")
                nc.vector.tensor_copy(vxs[:], pv[:])
                # transpose vxT slice -> vx [pos, dk] slots of vxa
                blk0 = p0 // 128
                for j in range(HPC):
                    vp = sc_ps.tile([128, 256], F32, tag="sc", name="vp")
                    for i in range(4):
                        nc.tensor.transpose(
                            vp[:, i * 64:(i + 1) * 64],
                            vxs[j * 64:(j + 1) * 64, i * 128:(i + 1) * 128],
                            t_id[j * 64:(j + 1) * 64, j * 64:(j + 1) * 64])
                    s0 = (j * 32 + blk0) * 64
                    nc.vector.tensor_copy(t_vxa[:, s0:s0 + 256], vp[:])
            warm_mm(1)

        def emit_slab(j, b, weave=None):
            """One (head, batch) slab: 16 merged steps, each = one path-A
            q-chunk (scores->exp->normalize->DMA) + two path-B k-chunk units
            (scores^T->exp->attn@v).  Merging keeps ACT, DMA and PE loaded
            simultaneously; B runs q-half 0 during steps 0-7, half 1 during
            8-15 so each half's accumulator can normalize and free early."""
            weave = weave or {}
            slab = j * B + b
            lo, hi = j * 64, (j + 1) * 64
            c0 = b * S
            sumsA = slabv.tile([128, QCH], F32, tag="sumsA", name="sumsA")
            recA = slabv.tile([128, QCH], F32, tag="recA", name="recA")
            po = [None, None]

            def finish_half(qh):
                # recA cols for this q-half -> [1, 1024] recips -> broadcast
                pt = sc_ps.tile([128, 1024], F32, tag="sc", name="pt")
                nc.tensor.transpose(pt[0:8, 0:128], recA[:, qh * 8:(qh + 1) * 8],
                                    t_id[:])
                rBt = slabv.tile([8, 128], F32, tag="rBt", name="rBt")
                nc.vector.tensor_copy(rBt[:], pt[0:8, 0:128])
                rB = slabv.tile([1, S // 2], F32, tag="rB", name="rB", bufs=2)
                nc.sync.dma_start(
                    rB[0:1, :].rearrange("a (c p) -> a c p", p=128), rBt[:])
                rbB = slabv.tile([64, S // 2], F32, tag="rbB", name="rbB", bufs=2)
                nc.gpsimd.partition_broadcast(rbB[:], rB[0:1, :])
                qb = c0 + qh * 1024
                dst = t_houtT[j][:, qb:qb + S // 2]
                nc.vector.tensor_mul(dst, po[qh][:], rbB[:])
                nc.vector.tensor_scalar(dst, dst, t_bv[j][:], None,
                                        op0=mybir.AluOpType.add)

            for s in range(16):
                qc = s
                qh = s // 8
                if s % 8 == 0:
                    po[qh] = o_ps.tile([64, S // 2], F32, tag="o", name="po")
                # ---- path A chunk ----
                et = e_pool.tile([128, S], F32, tag="E", name="et")
                sh = [misc.tile([128, 1], F32, tag="sh0", name="sh0"),
                      misc.tile([128, 1], F32, tag="sh1", name="sh1")]
                mq = t_mq[j][qc % 2]
                nc.vector.tensor_copy(mq[lo:hi, :],
                                      t_qxT[lo:hi, c0 + qc * 128: c0 + (qc + 1) * 128])
                for h in range(2):
                    ps = sc_ps.tile([128, 1024], F32, tag="sc", name="ps")
                    for ns in range(2):
                        nc.tensor.matmul(
                            ps[:, ns * 512:(ns + 1) * 512],
                            mq[:],
                            t_kxT[:, c0 + h * 1024 + ns * 512:
                                  c0 + h * 1024 + (ns + 1) * 512],
                            start=True, stop=True)
                    nc.scalar.activation(et[:, h * 1024:(h + 1) * 1024], ps[:],
                                         AF.Exp, scale=float(1.0 / TEMP),
                                         accum_out=sh[h][:])
                nc.vector.tensor_add(sumsA[:, qc:qc + 1], sh[0][:], sh[1][:])
                nc.vector.reciprocal(recA[:, qc:qc + 1], sumsA[:, qc:qc + 1])
                nc.vector.tensor_scalar_mul(et[:], et[:], recA[:, qc:qc + 1])
                nc.sync.dma_start(d_attn[slab, qc * 128:(qc + 1) * 128, :], et[:])

                # ---- two path-B k-chunk units (q-half qh) ----
                qb = c0 + qh * 1024
                for u in range(2):
                    kc = (s % 8) * 2 + u
                    ett = et_pool.tile([128, S // 2], F32R, tag="ET", name="ett")
                    mk = t_mk[j][kc % 2]
                    nc.vector.tensor_copy(
                        mk[lo:hi, :],
                        t_kxT[lo:hi, c0 + kc * 128: c0 + (kc + 1) * 128])
                    ps = scb_ps.tile([128, 1024], F32, tag="scb", name="ps")
                    for ns in range(2):
                        nc.tensor.matmul(
                            ps[:, ns * 512:(ns + 1) * 512],
                            mk[:],
                            t_qxT[:, qb + ns * 512: qb + (ns + 1) * 512],
                            start=True, stop=True)
                    nc.scalar.activation(ett[:], ps[:], AF.Exp,
                                         scale=float(1.0 / TEMP))
                    vslot = (j * 32 + b * 16 + kc) * 64
                    for qs in range(2):
                        nc.tensor.matmul(po[qh][:, qs * 512:(qs + 1) * 512],
                                         t_vxa[:, vslot:vslot + 64],
                                         ett[:, qs * 512:(qs + 1) * 512],
                                         start=(kc == 0), stop=(kc == KCH - 1))

                if s in weave:
                    weave[s]()
                elif s % 2 == 1:
                    warm_mm(1)
                if s % 8 == 7:
                    finish_half(qh)

        # minimal prefix: slab(0,0) needs all batch-0 k-slices, q-slices 0-1
        # (for A-chunks 0-7 and the B q-half 0); the rest weaves into slabs
        # so ACT starts exp work as early as possible
        for psl in range(4):
            emit_setup_half(0, psl, "k")
        emit_setup_half(0, 0, "q")
        emit_setup_half(0, 1, "q")
        emit_slab(0, 0,
                  weave={1: lambda: emit_setup_half(0, 2, "q"),
                         4: lambda: emit_setup_half(0, 3, "q"),
                         7: lambda: emit_setup_half(1, 0, "k"),
                         10: lambda: emit_setup_half(1, 0, "q"),
                         12: lambda: emit_setup_half(1, 1, "k"),
                         14: lambda: emit_setup_half(1, 1, "q")})
        emit_slab(1, 0,
                  weave={3: lambda: emit_setup_half(1, 2, "k"),
                         7: lambda: emit_setup_half(1, 2, "q"),
                         11: lambda: emit_setup_half(1, 3, "k"),
                         14: lambda: emit_setup_half(1, 3, "q")})
        for j in range(HPC):
            emit_slab(j, 1)

        for j in range(HPC):
            nc.sync.dma_start(d_houtT[j * 64:(j + 1) * 64, :], t_houtT[j][:])

    nc.compile()
    return nc


def _build_dense_module():
    RPC = POS // NCORES      # rows per core = 512
    nc = bacc.Bacc("TRN2", target_bir_lowering=False, debug=False,
                   enable_asserts=True, num_devices=NCORES)

    d_h = nc.dram_tensor("hout_t", (D, RPC), F32, kind="ExternalInput").ap()
    d_w = nc.dram_tensor("dense_wt", (D, D), F32, kind="ExternalInput").ap()
    d_bias = nc.dram_tensor("bias_b", (128, D), F32, kind="ExternalInput").ap()
    d_g = nc.dram_tensor("g_b", (128, D), F32, kind="ExternalInput").ap()
    d_lb = nc.dram_tensor("lb_b", (128, D), F32, kind="ExternalInput").ap()
    d_out = nc.dram_tensor("out2", (RPC, D), F32, kind="ExternalOutput").ap()

    with tile.TileContext(nc) as tc, ExitStack() as ctx:
        const = ctx.enter_context(tc.tile_pool(name="const", bufs=1))
        work = ctx.enter_context(tc.tile_pool(name="work", bufs=2))
        ps_p = ctx.enter_context(tc.tile_pool(name="ps", bufs=2, space="PSUM"))

        t_h = const.tile([128, 8 * RPC], F32R)     # chunk dc at cols dc*512
        nc.sync.dma_start(t_h[:].rearrange("p (a n) -> p a n", a=8),
                          d_h.rearrange("(a p) n -> p a n", p=128).bitcast(F32R))
        t_w = const.tile([128, 8 * D], F32R)       # chunk dc at cols dc*1024
        nc.sync.dma_start(t_w[:].rearrange("p (a n) -> p a n", a=8),
                          d_w.rearrange("(a p) n -> p a n", p=128).bitcast(F32R))
        t_bias = const.tile([128, D], F32)
        t_g = const.tile([128, D], F32)
        t_lb = const.tile([128, D], F32)
        t_eps = const.tile([128, 1], F32)
        nc.gpsimd.memset(t_eps[:], float(LN_EPS))
        nc.sync.dma_start(t_bias[:], d_bias)
        nc.sync.dma_start(t_g[:], d_g)
        nc.sync.dma_start(t_lb[:], d_lb)

        for rc in range(RPC // 128):
            pd = ps_p.tile([128, D], F32, tag="pd")
            for nch in range(2):
                for dc in range(8):
                    nc.tensor.matmul(
                        pd[:, nch * 512:(nch + 1) * 512],
                        t_h[:, dc * RPC + rc * 128: dc * RPC + (rc + 1) * 128],
                        t_w[:, dc * D + nch * 512: dc * D + (nch + 1) * 512],
                        start=(dc == 0), stop=(dc == 7))
            x = work.tile([128, D], F32, tag="x")
            nc.vector.tensor_add(x[:], pd[:], t_bias[:])
            x2 = work.tile([128, D], F32, tag="x2")
            s1 = work.tile([128, 1], F32, tag="s1")
            nc.scalar.activation(x2[:], x[:], AF.Relu, accum_out=s1[:])
            sq = work.tile([128, D], F32, tag="sq")
            s2 = work.tile([128, 1], F32, tag="s2")
            nc.scalar.activation(sq[:], x2[:], AF.Square, accum_out=s2[:])
            mu = work.tile([128, 1], F32, tag="mu")
            nc.vector.tensor_scalar_mul(mu[:], s1[:], float(1.0 / D))
            m2 = work.tile([128, 1], F32, tag="m2")
            nc.vector.tensor_scalar_mul(m2[:], s2[:], float(1.0 / D))
            mu2 = work.tile([128, 1], F32, tag="mu2")
            nc.vector.tensor_mul(mu2[:], mu[:], mu[:])
            var = work.tile([128, 1], F32, tag="var")
            nc.vector.tensor_sub(var[:], m2[:], mu2[:])
            sd = work.tile([128, 1], F32, tag="sd")
            nc.scalar.activation(sd[:], var[:], AF.Sqrt, bias=t_eps[:])
            rstd = work.tile([128, 1], F32, tag="rstd")
            nc.vector.reciprocal(rstd[:], sd[:])
            mb = work.tile([128, 1], F32, tag="mb")
            nc.vector.tensor_mul(mb[:], mu[:], rstd[:])
            xn = work.tile([128, D], F32, tag="xn")
            nc.vector.tensor_scalar(xn[:], x2[:], rstd[:], mb[:],
                                    op0=mybir.AluOpType.mult,
                                    op1=mybir.AluOpType.subtract)
            xg = work.tile([128, D], F32, tag="xg")
            nc.vector.tensor_mul(xg[:], xn[:], t_g[:])
            ot = work.tile([128, D], F32, tag="ot")
            nc.vector.tensor_add(ot[:], xg[:], t_lb[:])
            nc.sync.dma_start(d_out[rc * 128:(rc + 1) * 128, :], ot[:])

    nc.compile()
    return nc


_MODULES = {}
_LAST_IN_MAPS1 = None
_LAST_IN_MAPS2 = None


def _get_modules():
    if "attn" not in _MODULES:
        _MODULES["attn"] = _build_attn_module()
        _MODULES["dense"] = _build_dense_module()
    return _MODULES["attn"], _MODULES["dense"]


def kernel(k, q, w_k_w, w_k_b, w_q_w, w_q_b, w_v_w, w_v_b,
           dense_w, dense_b, ln_g, ln_b):
    k = np.asarray(k, np.float32)
    q = np.asarray(q, np.float32)
    w_k_w = np.asarray(w_k_w, np.float32)
    w_k_b = np.asarray(w_k_b, np.float32)
    w_q_w = np.asarray(w_q_w, np.float32)
    w_q_b = np.asarray(w_q_b, np.float32)
    w_v_w = np.asarray(w_v_w, np.float32)
    w_v_b = np.asarray(w_v_b, np.float32)
    dense_w = np.asarray(dense_w, np.float32)
    dense_b = np.asarray(dense_b, np.float32)
    ln_g = np.asarray(ln_g, np.float32)
    ln_b = np.asarray(ln_b, np.float32)

    nc1, nc2 = _get_modules()

    q2 = np.ascontiguousarray(q.reshape(POS, D))
    k2 = np.ascontiguousarray(k.reshape(POS, D))
    ident = np.eye(128, dtype=np.float32)

    in_maps1 = []
    for c in range(NCORES):
        sl = slice(c * HPC * DK, (c + 1) * HPC * DK)
        in_maps1.append(dict(
            q=q2, k=k2,
            wq_t=np.ascontiguousarray(w_q_w[sl].T),
            wk_t=np.ascontiguousarray(w_k_w[sl].T),
            wv_t=np.ascontiguousarray(w_v_w[sl].T),
            b_q=np.ascontiguousarray(w_q_b[sl].reshape(-1, 1)),
            b_k=np.ascontiguousarray(w_k_b[sl].reshape(-1, 1)),
            b_v=np.ascontiguousarray(w_v_b[sl].reshape(-1, 1)),
            ident=ident,
        ))
    global _LAST_IN_MAPS1
    _LAST_IN_MAPS1 = in_maps1
    res1 = run_bass_kernel_spmd(nc1, in_maps1, core_ids=list(range(NCORES)))
    attn = np.concatenate([r["attn"] for r in res1.results], axis=0)
    houtT = np.concatenate([r["houtT"] for r in res1.results], axis=0)  # [1024, 4096]

    dwt = np.ascontiguousarray(dense_w.T)
    bias_b = np.ascontiguousarray(np.broadcast_to(dense_b, (128, D)))
    g_b = np.ascontiguousarray(np.broadcast_to(ln_g, (128, D)))
    lb_b = np.ascontiguousarray(np.broadcast_to(ln_b, (128, D)))
    RPC = POS // NCORES
    in_maps2 = []
    for c in range(NCORES):
        in_maps2.append(dict(
            hout_t=np.ascontiguousarray(houtT[:, c * RPC:(c + 1) * RPC]),
            dense_wt=dwt, bias_b=bias_b, g_b=g_b, lb_b=lb_b,
        ))
    global _LAST_IN_MAPS2
    _LAST_IN_MAPS2 = in_maps2
    res2 = run_bass_kernel_spmd(nc2, in_maps2, core_ids=list(range(NCORES)))
    out = np.concatenate([r["out2"] for r in res2.results], axis=0).reshape(B, S, D)
    return out, attn
